# revision 1
# baseline (speedup 1.0000x reference)
"""GQA attention (B=2, S=1024, D=2048, 32 q heads / 8 kv heads, RoPE, causal)
on 8 TRN2 NeuronCores.

Strategy: pure data parallel — core c handles batch b = c // 4 and two
128-token blocks {j, 7-j} (j = c % 4) of that batch, which balances causal
attention work exactly (j+1 + 8-j = 9 kv-tiles per head for every core).
Each core computes full K/V for its batch (replicated within the 4-core
batch group), Q for its 256 tokens, attention, and its 256 rows of the
output projection. No collectives.

Layouts:
  - Q^T/K^T kept as [d, tok] (d on partitions) so scores^T[kt, qt] comes out
    of the PE directly with softmax's reduction (over kt) computable by
    matmul against a ones column appended to V.
  - RoPE done in deinterleaved space: Wq/Wk columns are permuted on the host
    (per-head even dims then odd dims), which leaves attention scores
    invariant; on device rope is t*C + swap(t)*D with host-built C/D tables
    and swap via a PE permutation matmul.
  - Causal mask applied multiplicatively (0/1, fractional on the diagonal
    tiles) to exp(scores) — mask content is per-core DATA so the SPMD
    instruction stream stays uniform.
"""

import numpy as np
import ml_dtypes

import concourse.bass as bass
import concourse.tile as tile
from concourse import bacc
from concourse import mybir
from concourse.bass_utils import run_bass_kernel_spmd

BF16 = ml_dtypes.bfloat16
D_MODEL = 2048
N_HEAD = 32
N_KV = 8
N_REP = 4
DK = 64
HALF = 32
THETA = 10000.0
B, S = 2, 1024
NT = S // 128  # 8 kv tiles of 128
QT = 256  # q tokens per core (two blocks of 128)

_cache = {}


def _build_nc(phases=3):
    nc = bacc.Bacc("TRN2", target_bir_lowering=False, debug=False)
    f32 = mybir.dt.float32
    bf16 = mybir.dt.bfloat16

    # ---- DRAM parameters (per-core shards supplied via in_maps) ----
    xT = nc.declare_dram_parameter("xT", [D_MODEL, S], bf16, isOutput=False)
    xqT = nc.declare_dram_parameter("xqT", [D_MODEL, QT], bf16, isOutput=False)
    wq = nc.declare_dram_parameter("wq", [D_MODEL, D_MODEL], bf16, isOutput=False)
    wk = nc.declare_dram_parameter("wk", [D_MODEL, 512], bf16, isOutput=False)
    wv = nc.declare_dram_parameter("wv", [D_MODEL, 512], bf16, isOutput=False)
    wo = nc.declare_dram_parameter("wo", [D_MODEL, D_MODEL], bf16, isOutput=False)
    bqr = nc.declare_dram_parameter("bqr", [1, D_MODEL], bf16, isOutput=False)
    bkr = nc.declare_dram_parameter("bkr", [1, 512], bf16, isOutput=False)
    bvr = nc.declare_dram_parameter("bvr", [1, 512], bf16, isOutput=False)
    bor = nc.declare_dram_parameter("bor", [1, D_MODEL], bf16, isOutput=False)
    ckt = nc.declare_dram_parameter("ckt", [128, S], bf16, isOutput=False)
    dkt = nc.declare_dram_parameter("dkt", [128, S], bf16, isOutput=False)
    cqt = nc.declare_dram_parameter("cqt", [128, QT], bf16, isOutput=False)
    dqt = nc.declare_dram_parameter("dqt", [128, QT], bf16, isOutput=False)
    pswap = nc.declare_dram_parameter("pswap", [128, 128], bf16, isOutput=False)
    packa = nc.declare_dram_parameter("packa", [64, 128], bf16, isOutput=False)
    packb = nc.declare_dram_parameter("packb", [64, 128], bf16, isOutput=False)
    onesb = nc.declare_dram_parameter("onesb", [65, 64], f32, isOutput=False)
    # mask[kt_local, i*256 + blk*128 + q_local] in {0, 1} (fractional = causal tri)
    maskT = nc.declare_dram_parameter("maskT", [128, NT * QT], bf16, isOutput=False)
    out = nc.declare_dram_parameter("out", [QT, D_MODEL], f32, isOutput=True)

    with tile.TileContext(nc) as tc:
        import contextlib

        with contextlib.ExitStack() as es:
            singles = es.enter_context(tc.tile_pool(name="singles", bufs=1))
            work = es.enter_context(tc.tile_pool(name="work", bufs=4))
            psA = es.enter_context(tc.tile_pool(name="psA", bufs=2, space="PSUM"))
            psB = es.enter_context(tc.tile_pool(name="psB", bufs=4, space="PSUM"))

            # ---- persistent constants / tables ----
            ck_sb = singles.tile([128, S], bf16)
            dk_sb = singles.tile([128, S], bf16)
            cq_sb = singles.tile([128, QT], bf16)
            dq_sb = singles.tile([128, QT], bf16)
            psw_sb = singles.tile([128, 128], bf16)
            pka_sb = singles.tile([64, 128], bf16)
            pkb_sb = singles.tile([64, 128], bf16)
            ones_sb = singles.tile([65, 64], f32)
            mask_sb = singles.tile([128, NT * QT], bf16)
            bq_sb = singles.tile([1, D_MODEL], bf16)
            bk_sb = singles.tile([1, 512], bf16)
            bv_sb = singles.tile([1, 512], bf16)
            bo_sb = singles.tile([1, D_MODEL], bf16)
            ones_row = singles.tile([1, 512], bf16)
            nc.vector.memset(ones_row, 1.0)
            for t, src in [
                (ck_sb, ckt), (dk_sb, dkt), (cq_sb, cqt), (dq_sb, dqt),
                (psw_sb, pswap), (pka_sb, packa), (pkb_sb, packb),
                (ones_sb, onesb), (mask_sb, maskT),
                (bq_sb, bqr), (bk_sb, bkr), (bv_sb, bvr), (bo_sb, bor),
            ]:
                nc.sync.dma_start(out=t, in_=src[:])
            # bq [2048] -> [128, 16]: col m holds bq[128m : 128m+128]

            # ---- persistent activations ----
            ropek = [singles.tile([64, S], bf16, name=f"ropek{i}", tag=f"ropek{i}") for i in range(N_KV)]
            ropeq = [singles.tile([64, QT], bf16, name=f"ropeq{i}", tag=f"ropeq{i}") for i in range(N_HEAD)]
            vp = [singles.tile([128, NT * 65], bf16, name=f"vp{i}", tag=f"vp{i}") for i in range(N_KV)]
            attT = [singles.tile([128, QT], bf16, name=f"attT{i}", tag=f"attT{i}") for i in range(N_HEAD // 2)]

            # =========== Phase 1: projections + rope ===========
            with contextlib.ExitStack() as proj_es:
                ppool = proj_es.enter_context(tc.tile_pool(name="proj", bufs=1))
                wqpool = proj_es.enter_context(tc.tile_pool(name="wqp", bufs=3))

                xT_sb = [ppool.tile([128, S], bf16, name=f"xt{kk}", tag=f"xt{kk}") for kk in range(16)]
                xq_sb = [ppool.tile([128, QT], bf16, name=f"xq{kk}", tag=f"xq{kk}") for kk in range(16)]
                wk_sb = [ppool.tile([128, 512], bf16, name=f"wk{kk}", tag=f"wk{kk}") for kk in range(16)]
                wv_sb = [ppool.tile([128, 512], bf16, name=f"wv{kk}", tag=f"wv{kk}") for kk in range(16)]
                for kk in range(16):
                    r = slice(kk * 128, kk * 128 + 128)
                    nc.sync.dma_start(out=xT_sb[kk], in_=xT[r, :])
                    nc.sync.dma_start(out=xq_sb[kk], in_=xqT[r, :])
                    nc.sync.dma_start(out=wk_sb[kk], in_=wk[r, :])
                    nc.sync.dma_start(out=wv_sb[kk], in_=wv[r, :])

                # ---- K^T = wk^T @ xT, rope -> ropek[64, S] per kv head ----
                for m in range(4):
                    kps = psA.tile([128, S], mybir.dt.float32, tag="A")
                    for hf in range(2):
                        cols = slice(hf * 512, hf * 512 + 512)
                        for kk in range(16):
                            nc.tensor.matmul(
                                kps[:, cols],
                                wk_sb[kk][:, m * 128:m * 128 + 128],
                                xT_sb[kk][:, cols],
                                start=(kk == 0), stop=False,
                            )
                        nc.tensor.matmul(
                            kps[:, cols],
                            bk_sb[:, m * 128:m * 128 + 128],
                            ones_row[:, 0:512],
                            start=False, stop=True,
                        )
                    k_sb = work.tile([128, S], bf16, tag="ksb")
                    nc.vector.tensor_copy(k_sb, kps)
                    swp = psA.tile([128, S], mybir.dt.float32, tag="A")
                    for hf in range(2):
                        cols = slice(hf * 512, hf * 512 + 512)
                        nc.tensor.matmul(swp[:, cols], psw_sb, k_sb[:, cols],
                                         start=True, stop=True)
                    t1 = work.tile([128, S], bf16, tag="t1")
                    t2 = work.tile([128, S], bf16, tag="t2")
                    nc.vector.tensor_mul(t1, k_sb, ck_sb)
                    nc.vector.tensor_mul(t2, swp, dk_sb)
                    nc.vector.tensor_add(ropek[2 * m], t1[0:64, :], t2[0:64, :])
                    nc.vector.tensor_add(ropek[2 * m + 1], t1[64:128, :], t2[64:128, :])

                # ---- V[t, dv] + bias -> vp tiles with ones column ----
                for h in range(N_KV):
                    nc.vector.memset(vp[h], 1.0)
                for i in range(NT):
                    vps = psA.tile([128, 512], mybir.dt.float32, tag="A")
                    for kk in range(16):
                        nc.tensor.matmul(
                            vps,
                            xT_sb[kk][:, i * 128:i * 128 + 128],
                            wv_sb[kk],
                            start=(kk == 0), stop=False,
                        )
                    nc.tensor.matmul(
                        vps,
                        ones_row[:, 0:128],
                        bv_sb,
                        start=False, stop=True,
                    )
                    for h in range(N_KV):
                        nc.vector.tensor_copy(
                            vp[h][:, i * 65:i * 65 + 64],
                            vps[:, h * 64:h * 64 + 64],
                        )

                # ---- Q^T = wq^T @ xqT, rope -> ropeq[64, QT] per head ----
                for m in range(16):
                    wqm = wqpool.tile([128, 16, 128], bf16, tag="wqm")
                    # wq[:, m*128 : m*128+128] laid out as [p, kk, c]
                    nc.sync.dma_start(
                        out=wqm,
                        in_=wq[:, m * 128:m * 128 + 128].rearrange(
                            "(kk p) c -> p kk c", p=128),
                    )
                    qps = psA.tile([128, QT], mybir.dt.float32, tag="A")
                    for kk in range(16):
                        nc.tensor.matmul(
                            qps,
                            wqm[:, kk, :],
                            xq_sb[kk],
                            start=(kk == 0), stop=False,
                        )
                    nc.tensor.matmul(
                        qps,
                        bq_sb[:, m * 128:m * 128 + 128],
                        ones_row[:, 0:QT],
                        start=False, stop=True,
                    )
                    q_sb = work.tile([128, QT], bf16, tag="qsb")
                    nc.vector.tensor_copy(q_sb, qps)
                    swq = psB.tile([128, QT], mybir.dt.float32, tag="B")
                    nc.tensor.matmul(swq, psw_sb, q_sb, start=True, stop=True)
                    t1 = work.tile([128, QT], bf16, tag="qt1")
                    t2 = work.tile([128, QT], bf16, tag="qt2")
                    nc.vector.tensor_mul(t1, q_sb, cq_sb)
                    nc.vector.tensor_mul(t2, swq, dq_sb)
                    nc.vector.tensor_add(ropeq[2 * m], t1[0:64, :], t2[0:64, :])
                    nc.vector.tensor_add(ropeq[2 * m + 1], t1[64:128, :], t2[64:128, :])

            # =========== Phase 2: attention ===========
            if phases < 2:
                return nc
            with contextlib.ExitStack() as att_es:
                apool = att_es.enter_context(tc.tile_pool(name="att", bufs=3))
                dpool = att_es.enter_context(tc.tile_pool(name="div", bufs=4))

                for pr in range(N_HEAD // 2):
                    attq = [None, None]
                    for sub in range(2):
                        h = 2 * pr + sub
                        kvh = h // N_REP
                        probs = apool.tile([128, NT * QT], bf16, tag="probs")
                        for half in range(2):
                            sps = psA.tile([128, 4 * QT], mybir.dt.float32, tag="A")
                            for ii in range(4):
                                i = half * 4 + ii
                                nc.tensor.matmul(
                                    sps[:, ii * QT:(ii + 1) * QT],
                                    ropek[kvh][:, i * 128:i * 128 + 128],
                                    ropeq[h],
                                    start=True, stop=True,
                                )
                            nc.scalar.activation(
                                probs[:, half * 4 * QT:(half + 1) * 4 * QT],
                                sps,
                                mybir.ActivationFunctionType.Exp,
                                bias=0.0, scale=0.125,
                            )
                        nc.vector.tensor_mul(probs, probs, mask_sb)
                        outv = psB.tile([65, QT], mybir.dt.float32, tag="B")
                        for i in range(NT):
                            nc.tensor.matmul(
                                outv,
                                vp[kvh][:, i * 65:i * 65 + 65],
                                probs[:, i * QT:(i + 1) * QT],
                                start=(i == 0), stop=(i == NT - 1),
                            )
                        rd = dpool.tile([65, QT], mybir.dt.float32, tag="rd")
                        nc.vector.reciprocal(rd[64:65, :], outv[64:65, :])
                        bcp = psB.tile([64, QT], mybir.dt.float32, tag="B")
                        nc.tensor.matmul(bcp, ones_sb[64:65, 0:64], rd[64:65, :],
                                         start=True, stop=True)
                        bc_sb = dpool.tile([64, QT], mybir.dt.float32, tag="bcs")
                        nc.vector.tensor_copy(bc_sb, bcp)
                        aq = dpool.tile([64, QT], bf16, tag=f"aq{sub}")
                        nc.vector.tensor_mul(aq, outv[0:64, :], bc_sb)
                        attq[sub] = aq
                    pk = psB.tile([128, QT], mybir.dt.float32, tag="B")
                    nc.tensor.matmul(pk, pka_sb, attq[0], start=True, stop=False)
                    nc.tensor.matmul(pk, pkb_sb, attq[1], start=False, stop=True)
                    nc.vector.tensor_copy(attT[pr], pk)

            # =========== Phase 3: output projection ===========
            if phases < 3:
                return nc
            with contextlib.ExitStack() as op_es:
                wopool = op_es.enter_context(tc.tile_pool(name="wop", bufs=2))
                opool = op_es.enter_context(tc.tile_pool(name="osb", bufs=3))
                for n in range(4):
                    won = wopool.tile([128, 16, 512], bf16, tag="won")
                    nc.sync.dma_start(
                        out=won,
                        in_=wo[:, n * 512:n * 512 + 512].rearrange(
                            "(p q) c -> q p c", q=128),
                    )
                    for blk in range(2):
                        ops = psA.tile([128, 512], mybir.dt.float32, tag="A")
                        for p in range(16):
                            nc.tensor.matmul(
                                ops,
                                attT[p][:, blk * 128:blk * 128 + 128],
                                won[:, p, :],
                                start=(p == 0), stop=False,
                            )
                        nc.tensor.matmul(
                            ops,
                            ones_row[:, 0:128],
                            bo_sb[:, n * 512:n * 512 + 512],
                            start=False, stop=True,
                        )
                        osb = opool.tile([128, 512], mybir.dt.float32, tag="osb")
                        nc.vector.tensor_copy(osb, ops)
                        nc.sync.dma_start(
                            out=out[blk * 128:blk * 128 + 128, n * 512:n * 512 + 512],
                            in_=osb,
                        )
    return nc


def _host_prep(x, Wq, bq, Wk, bk, Wv, bv, Wo, bo):
    """Build per-core input maps."""
    # per-head even/odd deinterleave permutation of output columns
    def colperm(nheads):
        p = []
        for h in range(nheads):
            base = h * DK
            p.extend([base + 2 * j for j in range(HALF)])
            p.extend([base + 2 * j + 1 for j in range(HALF)])
        return np.array(p)

    qperm = colperm(N_HEAD)
    kperm = colperm(N_KV)
    wq_p = np.ascontiguousarray(Wq[:, qperm]).astype(BF16)
    wk_p = np.ascontiguousarray(Wk[:, kperm]).astype(BF16)
    bq_p = np.ascontiguousarray(bq[qperm]).astype(BF16).reshape(1, D_MODEL)
    bk_p = np.ascontiguousarray(bk[kperm]).astype(BF16).reshape(1, 512)
    wv_c = Wv.astype(BF16)
    wo_c = Wo.astype(BF16)
    bv_r = bv.astype(BF16).reshape(1, 512)
    bo_r = bo.astype(BF16).reshape(1, D_MODEL)

    invf = THETA ** (-(np.arange(HALF, dtype=np.float64) * 2.0 / DK))
    posf = np.arange(S, dtype=np.float64)
    ang = posf[:, None] * invf[None, :]  # [S, 32]
    cos_t, sin_t = np.cos(ang), np.sin(ang)

    def rope_tables(pos_idx):
        # [128, len(pos_idx)] tables in deinterleaved space (2 heads / 128 rows)
        n = len(pos_idx)
        C = np.zeros((128, n), np.float32)
        D = np.zeros((128, n), np.float32)
        for p in range(128):
            r = p % DK
            i = r if r < HALF else r - HALF
            C[p] = cos_t[pos_idx, i]
            D[p] = (-sin_t[pos_idx, i]) if r < HALF else sin_t[pos_idx, i]
        return C.astype(BF16), D.astype(BF16)

    ckt, dkt = rope_tables(np.arange(S))

    psw = np.zeros((128, 128), np.float32)
    for m in range(128):
        k = m + HALF if (m % DK) < HALF else m - HALF
        psw[k, m] = 1.0
    psw = psw.astype(BF16)
    pka = np.zeros((64, 128), np.float32)
    pkb = np.zeros((64, 128), np.float32)
    for k in range(64):
        pka[k, k] = 1.0
        pkb[k, k + 64] = 1.0
    pka, pkb = pka.astype(BF16), pkb.astype(BF16)
    ones65 = np.ones((65, 64), np.float32)

    in_maps = []
    meta = []
    for c in range(8):
        b, j = c // 4, c % 4
        blocks = [j, 7 - j]
        qrows = np.concatenate([np.arange(bb * 128, bb * 128 + 128) for bb in blocks])
        xb = np.asarray(x[b], dtype=np.float32)
        xT = np.ascontiguousarray(xb.T).astype(BF16)
        xqT = np.ascontiguousarray(xb[qrows].T).astype(BF16)
        cqt, dqt = rope_tables(qrows)
        # mask[kt_local, i*QT + blk*128 + ql] = 1 if (i*128+kt_local) <= qpos else 0
        mask = np.zeros((128, NT * QT), np.float32)
        kt_local = np.arange(128)
        for i in range(NT):
            ktg = i * 128 + kt_local
            for blki, bb in enumerate(blocks):
                qpos = bb * 128 + np.arange(128)
                mask[:, i * QT + blki * 128:i * QT + blki * 128 + 128] = (
                    ktg[:, None] <= qpos[None, :]
                )
        in_maps.append({
            "xT": xT, "xqT": xqT, "wq": wq_p, "wk": wk_p, "wv": wv_c, "wo": wo_c,
            "bqr": bq_p, "bkr": bk_p, "bvr": bv_r, "bor": bo_r,
            "ckt": ckt, "dkt": dkt, "cqt": cqt, "dqt": dqt,
            "pswap": psw, "packa": pka, "packb": pkb, "onesb": ones65,
            "maskT": mask.astype(BF16),
        })
        meta.append((b, blocks))
    return in_maps, meta


def kernel(x, Wq, bq, Wk, bk, Wv, bv, Wo, bo):
    if "nc" not in _cache:
        nc0 = _build_nc()
        nc0.finalize()
        _cache["nc"] = nc0
    nc = _cache["nc"]
    in_maps, meta = _host_prep(x, Wq, bq, Wk, bk, Wv, bv, Wo, bo)
    res = run_bass_kernel_spmd(nc, in_maps, list(range(8)))
    full = np.zeros((B, S, D_MODEL), np.float32)
    for c in range(8):
        b, blocks = meta[c]
        o = res.results[c]["out"]
        for blki, bb in enumerate(blocks):
            full[b, bb * 128:bb * 128 + 128] = o[blki * 128:(blki + 1) * 128]
    return full



# revision 9
# speedup vs baseline: 1.2088x; 1.2088x over previous
"""GQA attention (B=2, S=1024, D=2048, 32 q heads / 8 kv heads, RoPE, causal)
on 8 TRN2 NeuronCores.

Strategy: data parallel on batch (4 cores per batch), with the K/V projection
sharded across the 4-core batch group and exchanged via AllGather (each core
projects only its 256 tokens, then gathers the full 1024-token K/V). Core c
handles batch b = c // 4 and two 128-token q blocks {j, 7-j} (j = c % 4),
which balances causal-mask waste exactly across cores.

Layouts:
  - Q^T/K^T kept as [d, tok] (d on partitions) so scores^T[kt, qt] comes out
    of the PE directly; softmax's reduction (over kt) is computed by matmul
    against a ones column appended to V (the 65-row AV trick).
  - RoPE done in deinterleaved space: Wq/Wk columns are permuted on the host
    (per-head even dims then odd dims); rope is t*C + swap(t)*D with
    host-built C/D tables and swap via a PE permutation matmul.
  - KV tiles are stored in rank-slot order after the AllGather; the causal
    mask (per-core DATA, so the SPMD instruction stream stays uniform)
    absorbs the reordering.
  - All 32 heads' softmax denominators are packed into one [32, 256] PSUM
    tile via K=1 matmuls and inverted with a single reciprocal_approx_fast
    (the per-head DVE reciprocal was the #1 vector-engine hotspot).
"""

import numpy as np
import ml_dtypes

import concourse.bass as bass
import concourse.tile as tile
from concourse import bacc
from concourse import mybir
from concourse.bass_utils import run_bass_kernel_spmd

BF16 = ml_dtypes.bfloat16
D_MODEL = 2048
N_HEAD = 32
N_KV = 8
N_REP = 4
DK = 64
HALF = 32
THETA = 10000.0
B, S = 2, 1024
NT = 8           # kv tiles of 128
QT = 256         # q tokens per core (two blocks of 128)
NPAIR = 16       # head pairs
RG = [[0, 1, 2, 3], [4, 5, 6, 7]]

_cache = {}


def _build_nc():
    nc = bacc.Bacc("TRN2", target_bir_lowering=False, debug=False, num_devices=8)
    f32 = mybir.dt.float32
    bf16 = mybir.dt.bfloat16

    # ---- DRAM parameters (per-core shards supplied via in_maps) ----
    xqT = nc.declare_dram_parameter("xqT", [D_MODEL, QT], bf16, isOutput=False)
    wq = nc.declare_dram_parameter("wq", [D_MODEL, D_MODEL], bf16, isOutput=False)
    wk = nc.declare_dram_parameter("wk", [D_MODEL, 512], bf16, isOutput=False)
    wv = nc.declare_dram_parameter("wv", [D_MODEL, 512], bf16, isOutput=False)
    wo = nc.declare_dram_parameter("wo", [D_MODEL, D_MODEL], bf16, isOutput=False)
    bqr = nc.declare_dram_parameter("bqr", [1, D_MODEL], bf16, isOutput=False)
    bkr = nc.declare_dram_parameter("bkr", [1, 512], bf16, isOutput=False)
    bvr = nc.declare_dram_parameter("bvr", [1, 512], bf16, isOutput=False)
    bor = nc.declare_dram_parameter("bor", [1, D_MODEL], bf16, isOutput=False)
    cqt = nc.declare_dram_parameter("cqt", [128, QT], bf16, isOutput=False)
    dqt = nc.declare_dram_parameter("dqt", [128, QT], bf16, isOutput=False)
    pswap = nc.declare_dram_parameter("pswap", [128, 128], bf16, isOutput=False)
    packa = nc.declare_dram_parameter("packa", [64, 128], bf16, isOutput=False)
    packb = nc.declare_dram_parameter("packb", [64, 128], bf16, isOutput=False)
    dsel = nc.declare_dram_parameter("dsel", [65, N_HEAD * 32], bf16, isOutput=False)
    selp = nc.declare_dram_parameter("selp", [32, NPAIR * 128], bf16, isOutput=False)
    # mask[kt, t*256 + blk*128 + ql] in {0, 1} (fractional = causal tri),
    # kv tile t in rank-slot order
    maskT = nc.declare_dram_parameter("maskT", [128, NT * QT], bf16, isOutput=False)
    out = nc.declare_dram_parameter("out", [QT, D_MODEL], f32, isOutput=True)

    with tile.TileContext(nc) as tc:
        import contextlib

        with contextlib.ExitStack() as es:
            singles = es.enter_context(tc.tile_pool(name="singles", bufs=1))
            dram = es.enter_context(tc.tile_pool(name="dram", bufs=1, space="DRAM"))

            # ---- persistent constants / tables ----
            cq_sb = singles.tile([128, QT], bf16)
            dq_sb = singles.tile([128, QT], bf16)
            psw_sb = singles.tile([128, 128], bf16)
            pka_sb = singles.tile([64, 128], bf16)
            pkb_sb = singles.tile([64, 128], bf16)
            dsel_sb = singles.tile([65, N_HEAD * 32], bf16)
            selp_sb = singles.tile([32, NPAIR * 128], bf16)
            mask_sb = singles.tile([128, NT, QT], bf16)
            bq_sb = singles.tile([1, D_MODEL], bf16)
            bk_sb = singles.tile([1, 512], bf16)
            bv_sb = singles.tile([1, 512], bf16)
            bo_sb = singles.tile([1, D_MODEL], bf16)
            ones_row = singles.tile([1, 512], bf16)
            nc.vector.memset(ones_row, 1.0)
            for t, src in [
                (cq_sb, cqt), (dq_sb, dqt), (psw_sb, pswap),
                (pka_sb, packa), (pkb_sb, packb),
                (dsel_sb, dsel), (selp_sb, selp),
                (bq_sb, bqr), (bk_sb, bkr), (bv_sb, bvr), (bo_sb, bor),
            ]:
                nc.sync.dma_start(out=t, in_=src[:])
            nc.sync.dma_start(
                out=mask_sb, in_=maskT[:].rearrange("p (t c) -> p t c", t=NT))

            # ---- DRAM bounce buffers for the K/V AllGather ----
            k_in = dram.tile([512, QT], bf16, name="k_in", tag="k_in")
            k_out = dram.tile([4 * 512, QT], bf16, name="k_out", tag="k_out")
            v_in = dram.tile([QT, N_KV * 65], bf16, name="v_in", tag="v_in")
            v_out = dram.tile([4 * QT, N_KV * 65], bf16, name="v_out", tag="v_out")

            # ---- persistent activations ----
            ropek2 = [singles.tile([64, 4, QT], bf16, name=f"ropek{i}", tag=f"ropek{i}")
                      for i in range(N_KV)]
            ropeq = [singles.tile([64, QT], bf16, name=f"ropeq{i}", tag=f"ropeq{i}")
                     for i in range(N_HEAD)]
            vall = [singles.tile([128, N_KV * 65], bf16, name=f"vall{i}", tag=f"vall{i}")
                    for i in range(NT)]
            attT_raw = [singles.tile([128, QT], bf16, name=f"attR{i}", tag=f"attR{i}")
                        for i in range(NPAIR)]
            attT = [singles.tile([128, QT], bf16, name=f"attT{i}", tag=f"attT{i}")
                    for i in range(NPAIR)]
            rec_sb = singles.tile([32, QT], mybir.dt.float32)
            rec_bf = singles.tile([32, QT], bf16)
            xq_sb = [singles.tile([128, QT], bf16, name=f"xq{kk}", tag=f"xq{kk}")
                     for kk in range(16)]
            for kk in range(16):
                nc.sync.dma_start(out=xq_sb[kk], in_=xqT[kk * 128:kk * 128 + 128, :])

            # =========== Phase 1a: K/V projection (my 256 tokens) + AG ======
            with contextlib.ExitStack() as kv_es:
                kvp = kv_es.enter_context(tc.tile_pool(name="kvp", bufs=1))
                kwork = kv_es.enter_context(tc.tile_pool(name="kwork", bufs=4))
                psA = kv_es.enter_context(tc.tile_pool(name="psA", bufs=2, space="PSUM"))
                psB = kv_es.enter_context(tc.tile_pool(name="psB", bufs=2, space="PSUM"))

                wk_sb = [kvp.tile([128, 512], bf16, name=f"wk{kk}", tag=f"wk{kk}")
                         for kk in range(16)]
                wv_sb = [kvp.tile([128, 512], bf16, name=f"wv{kk}", tag=f"wv{kk}")
                         for kk in range(16)]
                for kk in range(16):
                    r = slice(kk * 128, kk * 128 + 128)
                    nc.sync.dma_start(out=wk_sb[kk], in_=wk[r, :])
                    nc.sync.dma_start(out=wv_sb[kk], in_=wv[r, :])

                # K^T = wk^T @ xqT + bk, rope, -> k_in rows m*128..
                for m in range(4):
                    kps = psA.tile([128, QT], mybir.dt.float32, tag="A")
                    for kk in range(16):
                        nc.tensor.matmul(
                            kps, wk_sb[kk][:, m * 128:m * 128 + 128], xq_sb[kk],
                            start=(kk == 0), stop=False)
                    nc.tensor.matmul(
                        kps, bk_sb[:, m * 128:m * 128 + 128], ones_row[:, 0:QT],
                        start=False, stop=True)
                    k_sb = kwork.tile([128, QT], bf16, tag="ksb")
                    nc.vector.tensor_copy(k_sb, kps)
                    swp = psB.tile([128, QT], mybir.dt.float32, tag="B")
                    nc.tensor.matmul(swp, psw_sb, k_sb, start=True, stop=True)
                    t1 = kwork.tile([128, QT], bf16, tag="t1")
                    t2 = kwork.tile([128, QT], bf16, tag="t2")
                    kro = kwork.tile([128, QT], bf16, tag="kro")
                    nc.vector.tensor_mul(t1, k_sb, cq_sb)
                    nc.vector.tensor_mul(t2, swp, dq_sb)
                    nc.vector.tensor_add(kro, t1, t2)
                    nc.sync.dma_start(out=k_in[m * 128:m * 128 + 128, :], in_=kro)
                nc.gpsimd.collective_compute(
                    "AllGather", mybir.AluOpType.bypass, replica_groups=RG,
                    ins=[k_in.opt()], outs=[k_out.opt()])

                # V = xq @ wv + bv (token-major, ones col per kv head) -> v_in
                for blk in range(2):
                    vps = psA.tile([128, 512], mybir.dt.float32, tag="A")
                    for kk in range(16):
                        nc.tensor.matmul(
                            vps, xq_sb[kk][:, blk * 128:blk * 128 + 128], wv_sb[kk],
                            start=(kk == 0), stop=False)
                    nc.tensor.matmul(
                        vps, ones_row[:, 0:128], bv_sb, start=False, stop=True)
                    vloc = kwork.tile([128, N_KV * 65], bf16, tag="vloc")
                    nc.vector.memset(vloc, 1.0)
                    for h in range(N_KV):
                        nc.vector.tensor_copy(
                            vloc[:, h * 65:h * 65 + 64], vps[:, h * 64:h * 64 + 64])
                    nc.sync.dma_start(out=v_in[blk * 128:blk * 128 + 128, :], in_=vloc)
                nc.gpsimd.collective_compute(
                    "AllGather", mybir.AluOpType.bypass, replica_groups=RG,
                    ins=[v_in.opt()], outs=[v_out.opt()])

            # =========== Phase 1b: Q projection + rope ===========
            with contextlib.ExitStack() as q_es:
                qwork = q_es.enter_context(tc.tile_pool(name="qwork", bufs=4))
                wqpool = q_es.enter_context(tc.tile_pool(name="wqp", bufs=3))
                psQ = q_es.enter_context(tc.tile_pool(name="psQ", bufs=2, space="PSUM"))
                psR = q_es.enter_context(tc.tile_pool(name="psR", bufs=2, space="PSUM"))

                for m in range(16):
                    wqm = wqpool.tile([128, 16, 128], bf16, tag="wqm")
                    nc.sync.dma_start(
                        out=wqm,
                        in_=wq[:, m * 128:m * 128 + 128].rearrange(
                            "(kk p) c -> p kk c", p=128))
                    qps = psQ.tile([128, QT], mybir.dt.float32, tag="Q")
                    for kk in range(16):
                        nc.tensor.matmul(
                            qps, wqm[:, kk, :], xq_sb[kk],
                            start=(kk == 0), stop=False)
                    nc.tensor.matmul(
                        qps, bq_sb[:, m * 128:m * 128 + 128], ones_row[:, 0:QT],
                        start=False, stop=True)
                    q_sb = qwork.tile([128, QT], bf16, tag="qsb")
                    nc.vector.tensor_copy(q_sb, qps)
                    swq = psR.tile([128, QT], mybir.dt.float32, tag="R")
                    nc.tensor.matmul(swq, psw_sb, q_sb, start=True, stop=True)
                    t1 = qwork.tile([128, QT], bf16, tag="qt1")
                    t2 = qwork.tile([128, QT], bf16, tag="qt2")
                    nc.vector.tensor_mul(t1, q_sb, cq_sb)
                    nc.vector.tensor_mul(t2, swq, dq_sb)
                    nc.vector.tensor_add(ropeq[2 * m], t1[0:64, :], t2[0:64, :])
                    nc.vector.tensor_add(ropeq[2 * m + 1], t1[64:128, :], t2[64:128, :])

            # ---- load gathered K/V into SBUF (rank-slot kv tile order) ----
            kog = k_out[:].rearrange("(r kh p) c -> kh p r c", r=4, kh=N_KV, p=64)
            for kvh in range(N_KV):
                nc.sync.dma_start(out=ropek2[kvh], in_=kog[kvh])
            vog = v_out[:].rearrange("(r s p) c -> (r s) p c", r=4, s=2, p=128)
            for t in range(NT):
                nc.sync.dma_start(out=vall[t], in_=vog[t])

            # =========== Phase 2: attention ===========
            with contextlib.ExitStack() as att_es:
                probsp = att_es.enter_context(tc.tile_pool(name="probsp", bufs=2))
                unp = att_es.enter_context(tc.tile_pool(name="unp", bufs=2))
                psS = att_es.enter_context(tc.tile_pool(name="psS", bufs=2, space="PSUM"))
                psV = att_es.enter_context(tc.tile_pool(name="psV", bufs=1, space="PSUM"))
                psD = att_es.enter_context(tc.tile_pool(name="psD", bufs=1, space="PSUM"))
                psM = att_es.enter_context(tc.tile_pool(name="psM", bufs=2, space="PSUM"))
                wopool = att_es.enter_context(tc.tile_pool(name="wop", bufs=2))
                opool = att_es.enter_context(tc.tile_pool(name="osb", bufs=3))

                psd = psD.tile([32, QT], mybir.dt.float32, tag="D")

                for pr in range(NPAIR):
                    kvh = pr // 2
                    probs = probsp.tile([128, NT, 2, QT], bf16, tag="P")
                    for sub in range(2):
                        h = 2 * pr + sub
                        for half in range(2):
                            pss = psS.tile([128, 4, QT], mybir.dt.float32, tag="S")
                            for tt in range(4):
                                t = half * 4 + tt
                                nc.tensor.matmul(
                                    pss[:, tt, :],
                                    ropek2[kvh][:, t // 2, (t % 2) * 128:(t % 2) * 128 + 128],
                                    ropeq[h],
                                    start=True, stop=True)
                            pv = probs[:, half * 4:half * 4 + 4, sub, :]
                            nc.scalar.activation(
                                pv, pss,
                                mybir.ActivationFunctionType.Exp,
                                bias=0.0, scale=0.125)
                            nc.vector.tensor_mul(
                                pv, pv, mask_sb[:, half * 4:half * 4 + 4, :])
                    # AV with pair-shared stationary V (N=512: [A|B])
                    psv = psV.tile([65, 512], mybir.dt.float32, tag="V")
                    for t in range(NT):
                        nc.tensor.matmul(
                            psv, vall[t][:, kvh * 65:kvh * 65 + 65],
                            probs[:, t, :, :],
                            start=(t == 0), stop=(t == NT - 1))
                    unnorm = unp.tile([65, 512], bf16, tag="U")
                    nc.vector.tensor_copy(unnorm, psv)
                    # denominators (row 64) -> psd rows {2pr, 2pr+1}
                    for sub in range(2):
                        h = 2 * pr + sub
                        nc.tensor.matmul(
                            psd, dsel_sb[64:65, h * 32:h * 32 + 32],
                            unnorm[64:65, sub * QT:sub * QT + QT],
                            start=(pr == 0 and sub == 0),
                            stop=(pr == NPAIR - 1 and sub == 1))
                    # pack the two heads' [64, 256] into [128, 256]
                    pk = psM.tile([128, 512], mybir.dt.float32, tag="M")
                    nc.tensor.matmul(pk[:, 0:QT], pka_sb, unnorm[0:64, 0:QT],
                                     start=True, stop=False)
                    nc.tensor.matmul(pk[:, 0:QT], pkb_sb, unnorm[0:64, QT:512],
                                     start=False, stop=True)
                    nc.vector.tensor_copy(attT_raw[pr], pk[:, 0:QT])

                # one batched reciprocal for all 32 heads' denominators
                nc.vector.reciprocal_approx_fast(rec_sb, psd)
                nc.vector.tensor_copy(rec_bf, rec_sb)
                for pr in range(NPAIR):
                    bcp = psM.tile([128, 512], mybir.dt.float32, tag="M")
                    nc.tensor.matmul(
                        bcp[:, 0:QT], selp_sb[:, pr * 128:pr * 128 + 128], rec_bf,
                        start=True, stop=True)
                    nc.vector.tensor_mul(attT[pr], attT_raw[pr], bcp[:, 0:QT])

                # =========== Phase 3: output projection ===========
                for n in range(4):
                    won = wopool.tile([128, 16, 512], bf16, tag="won")
                    nc.sync.dma_start(
                        out=won,
                        in_=wo[:, n * 512:n * 512 + 512].rearrange(
                            "(p q) c -> q p c", q=128))
                    for blk in range(2):
                        ops = psM.tile([128, 512], mybir.dt.float32, tag="M")
                        for p in range(16):
                            nc.tensor.matmul(
                                ops, attT[p][:, blk * 128:blk * 128 + 128],
                                won[:, p, :],
                                start=(p == 0), stop=False)
                        nc.tensor.matmul(
                            ops, ones_row[:, 0:128], bo_sb[:, n * 512:n * 512 + 512],
                            start=False, stop=True)
                        osb = opool.tile([128, 512], mybir.dt.float32, tag="osb")
                        nc.vector.tensor_copy(osb, ops)
                        nc.sync.dma_start(
                            out=out[blk * 128:blk * 128 + 128, n * 512:n * 512 + 512],
                            in_=osb)
    return nc


def _host_prep(x, Wq, bq, Wk, bk, Wv, bv, Wo, bo):
    """Build per-core input maps."""
    # per-head even/odd deinterleave permutation of output columns
    def colperm(nheads):
        p = []
        for h in range(nheads):
            base = h * DK
            p.extend([base + 2 * j for j in range(HALF)])
            p.extend([base + 2 * j + 1 for j in range(HALF)])
        return np.array(p)

    qperm = colperm(N_HEAD)
    kperm = colperm(N_KV)
    wq_p = np.ascontiguousarray(Wq[:, qperm]).astype(BF16)
    wk_p = np.ascontiguousarray(Wk[:, kperm]).astype(BF16)
    bq_p = np.ascontiguousarray(bq[qperm]).astype(BF16).reshape(1, D_MODEL)
    bk_p = np.ascontiguousarray(bk[kperm]).astype(BF16).reshape(1, 512)
    wv_c = Wv.astype(BF16)
    wo_c = Wo.astype(BF16)
    bv_r = bv.astype(BF16).reshape(1, 512)
    bo_r = bo.astype(BF16).reshape(1, D_MODEL)

    invf = THETA ** (-(np.arange(HALF, dtype=np.float64) * 2.0 / DK))
    posf = np.arange(S, dtype=np.float64)
    ang = posf[:, None] * invf[None, :]  # [S, 32]
    cos_t, sin_t = np.cos(ang), np.sin(ang)

    def rope_tables(pos_idx):
        # [128, len(pos_idx)] tables in deinterleaved space (2 heads / 128 rows)
        n = len(pos_idx)
        C = np.zeros((128, n), np.float32)
        D = np.zeros((128, n), np.float32)
        for p in range(128):
            r = p % DK
            i = r if r < HALF else r - HALF
            C[p] = cos_t[pos_idx, i]
            D[p] = (-sin_t[pos_idx, i]) if r < HALF else sin_t[pos_idx, i]
        return C.astype(BF16), D.astype(BF16)

    psw = np.zeros((128, 128), np.float32)
    for m in range(128):
        k = m + HALF if (m % DK) < HALF else m - HALF
        psw[k, m] = 1.0
    psw = psw.astype(BF16)
    pka = np.zeros((64, 128), np.float32)
    pkb = np.zeros((64, 128), np.float32)
    for k in range(64):
        pka[k, k] = 1.0
        pkb[k, k + 64] = 1.0
    pka, pkb = pka.astype(BF16), pkb.astype(BF16)

    dsel = np.zeros((65, N_HEAD * 32), np.float32)
    for h in range(N_HEAD):
        dsel[64, h * 32 + (h % 32)] = 1.0
    dsel = dsel.astype(BF16)
    selp = np.zeros((32, NPAIR * 128), np.float32)
    for pr in range(NPAIR):
        selp[2 * pr, pr * 128:pr * 128 + 64] = 1.0
        selp[2 * pr + 1, pr * 128 + 64:pr * 128 + 128] = 1.0
    selp = selp.astype(BF16)

    # kv tile t (rank-slot order) covers q block: r if slot 0 else 7-r
    kvblock = [(t // 2) if (t % 2 == 0) else 7 - (t // 2) for t in range(NT)]

    in_maps = []
    meta = []
    for c in range(8):
        b, j = c // 4, c % 4
        blocks = [j, 7 - j]
        qrows = np.concatenate([np.arange(bb * 128, bb * 128 + 128) for bb in blocks])
        xb = np.asarray(x[b], dtype=np.float32)
        xqT = np.ascontiguousarray(xb[qrows].T).astype(BF16)
        cqt, dqt = rope_tables(qrows)
        # mask[kt, t*QT + blk*128 + ql] = 1 if kv_pos <= q_pos else 0
        mask = np.zeros((128, NT * QT), np.float32)
        kt_local = np.arange(128)
        for t in range(NT):
            ktg = kvblock[t] * 128 + kt_local
            for blki, bb in enumerate(blocks):
                qpos = bb * 128 + np.arange(128)
                mask[:, t * QT + blki * 128:t * QT + blki * 128 + 128] = (
                    ktg[:, None] <= qpos[None, :]
                )
        in_maps.append({
            "xqT": xqT, "wq": wq_p, "wk": wk_p, "wv": wv_c, "wo": wo_c,
            "bqr": bq_p, "bkr": bk_p, "bvr": bv_r, "bor": bo_r,
            "cqt": cqt, "dqt": dqt,
            "pswap": psw, "packa": pka, "packb": pkb,
            "dsel": dsel, "selp": selp,
            "maskT": mask.astype(BF16),
        })
        meta.append((b, blocks))
    return in_maps, meta


def kernel(x, Wq, bq, Wk, bk, Wv, bv, Wo, bo):
    if "nc" not in _cache:
        nc0 = _build_nc()
        nc0.finalize()
        _cache["nc"] = nc0
    nc = _cache["nc"]
    in_maps, meta = _host_prep(x, Wq, bq, Wk, bk, Wv, bv, Wo, bo)
    res = run_bass_kernel_spmd(nc, in_maps, list(range(8)))
    full = np.zeros((B, S, D_MODEL), np.float32)
    for c in range(8):
        b, blocks = meta[c]
        o = res.results[c]["out"]
        for blki, bb in enumerate(blocks):
            full[b, bb * 128:bb * 128 + 128] = o[blki * 128:(blki + 1) * 128]
    return full


# revision 14
# speedup vs baseline: 1.2448x; 1.0298x over previous
"""GQA attention (B=2, S=1024, D=2048, 32 q heads / 8 kv heads, RoPE, causal)
on 8 TRN2 NeuronCores.

Strategy: data parallel on batch (4 cores per batch), with the K/V projection
sharded across the 4-core batch group and exchanged via AllGather (each core
projects only its 256 tokens, then gathers the full 1024-token K/V). Core c
handles batch b = c // 4 and two 128-token q blocks {j, 7-j} (j = c % 4),
which balances causal-mask waste exactly across cores.

Layouts:
  - Q^T/K^T kept as [d, tok] (d on partitions) so scores^T[kt, qt] comes out
    of the PE directly; softmax's reduction (over kt) is computed by matmul
    against a ones column appended to V (the 65-row AV trick).
  - RoPE done in deinterleaved space: Wq/Wk columns are permuted on the host
    (per-head even dims then odd dims); rope is t*C + swap(t)*D with
    host-built C/D tables and swap via a PE permutation matmul.
  - KV tiles are stored in rank-slot order after the AllGather; the causal
    mask (per-core DATA, so the SPMD instruction stream stays uniform)
    absorbs the reordering.
  - Softmax denominators are collected into a [32, 256] SBUF tile via
    partition-shifted DVE copies and inverted with two batched
    reciprocal_approx_fast calls (per-head DVE reciprocal was the #1
    vector-engine hotspot in the original kernel).
  - DMAs are spread across engine queues (sync/vector/scalar/gpsimd) so
    weight streaming, collective bounces, and output stores don't serialize.
"""

import numpy as np
import ml_dtypes

import concourse.bass as bass
import concourse.tile as tile
from concourse import bacc
from concourse import mybir
from concourse.bass_utils import run_bass_kernel_spmd

BF16 = ml_dtypes.bfloat16
D_MODEL = 2048
N_HEAD = 32
N_KV = 8
N_REP = 4
DK = 64
HALF = 32
THETA = 10000.0
B, S = 2, 1024
NT = 8           # kv tiles of 128
QT = 256         # q tokens per core (two blocks of 128)
NPAIR = 16       # head pairs
RG = [[0, 1, 2, 3], [4, 5, 6, 7]]

_cache = {}


def _build_nc():
    nc = bacc.Bacc("TRN2", target_bir_lowering=False, debug=False, num_devices=8)
    f32 = mybir.dt.float32
    bf16 = mybir.dt.bfloat16

    # ---- DRAM parameters (per-core shards supplied via in_maps) ----
    xqT = nc.declare_dram_parameter("xqT", [D_MODEL, QT], bf16, isOutput=False)
    wq = nc.declare_dram_parameter("wq", [D_MODEL, D_MODEL], bf16, isOutput=False)
    wk = nc.declare_dram_parameter("wk", [D_MODEL, 512], bf16, isOutput=False)
    wv = nc.declare_dram_parameter("wv", [D_MODEL, 512], bf16, isOutput=False)
    wo = nc.declare_dram_parameter("wo", [D_MODEL, D_MODEL], bf16, isOutput=False)
    bqr = nc.declare_dram_parameter("bqr", [1, D_MODEL], bf16, isOutput=False)
    bkr = nc.declare_dram_parameter("bkr", [1, 512], bf16, isOutput=False)
    bvr = nc.declare_dram_parameter("bvr", [1, 512], bf16, isOutput=False)
    bor = nc.declare_dram_parameter("bor", [1, D_MODEL], bf16, isOutput=False)
    cqt = nc.declare_dram_parameter("cqt", [128, QT], bf16, isOutput=False)
    dqt = nc.declare_dram_parameter("dqt", [128, QT], bf16, isOutput=False)
    pswap = nc.declare_dram_parameter("pswap", [128, 128], bf16, isOutput=False)
    packa = nc.declare_dram_parameter("packa", [64, 128], bf16, isOutput=False)
    packb = nc.declare_dram_parameter("packb", [64, 128], bf16, isOutput=False)
    dsel = nc.declare_dram_parameter("dsel", [65, N_HEAD * 16], bf16, isOutput=False)
    selp = nc.declare_dram_parameter("selp", [16, NPAIR * 128], bf16, isOutput=False)
    # mask[kt, t*256 + blk*128 + ql] in {0, 1} (fractional = causal tri),
    # kv tile t in rank-slot order
    maskT = nc.declare_dram_parameter("maskT", [128, NT * QT], bf16, isOutput=False)
    out = nc.declare_dram_parameter("out", [QT, D_MODEL], f32, isOutput=True)

    with tile.TileContext(nc) as tc:
        import contextlib

        with contextlib.ExitStack() as es:
            singles = es.enter_context(tc.tile_pool(name="singles", bufs=1))
            dram = es.enter_context(tc.tile_pool(name="dram", bufs=1, space="DRAM"))

            # ---- persistent constants / tables (vector DMA queue) ----
            cq_sb = singles.tile([128, QT], bf16)
            dq_sb = singles.tile([128, QT], bf16)
            psw_sb = singles.tile([128, 128], bf16)
            pka_sb = singles.tile([64, 128], bf16)
            pkb_sb = singles.tile([64, 128], bf16)
            dsel_sb = singles.tile([65, N_HEAD * 16], bf16)
            selp_sb = singles.tile([16, NPAIR * 128], bf16)
            mask_sb = singles.tile([128, NT, QT], bf16)
            bq_sb = singles.tile([1, D_MODEL], bf16)
            bk_sb = singles.tile([1, 512], bf16)
            bv_sb = singles.tile([1, 512], bf16)
            bo_sb = singles.tile([1, D_MODEL], bf16)
            ones_row = singles.tile([1, 512], bf16)
            nc.vector.memset(ones_row, 1.0)
            for t, src in [
                (cq_sb, cqt), (dq_sb, dqt), (psw_sb, pswap),
                (pka_sb, packa), (pkb_sb, packb), (dsel_sb, dsel), (selp_sb, selp),
                (bq_sb, bqr), (bk_sb, bkr), (bv_sb, bvr), (bo_sb, bor),
            ]:
                nc.scalar.dma_start(out=t, in_=src[:])
            nc.scalar.dma_start(
                out=mask_sb, in_=maskT[:].rearrange("p (t c) -> p t c", t=NT))

            # ---- DRAM bounce buffers for the K/V AllGather ----
            k_in = dram.tile([512, QT], bf16, name="k_in", tag="k_in")
            k_out = dram.tile([4 * 512, QT], bf16, name="k_out", tag="k_out")
            v_in = dram.tile([QT, N_KV * 65], bf16, name="v_in", tag="v_in")
            v_out = dram.tile([4 * QT, N_KV * 65], bf16, name="v_out", tag="v_out")

            # ---- persistent activations ----
            ropek2 = [singles.tile([64, 4, QT], bf16, name=f"ropek{i}", tag=f"ropek{i}")
                      for i in range(N_KV)]
            ropeq = [singles.tile([64, QT], bf16, name=f"ropeq{i}", tag=f"ropeq{i}")
                     for i in range(N_HEAD)]
            vall = [singles.tile([128, N_KV * 65], bf16, name=f"vall{i}", tag=f"vall{i}")
                    for i in range(NT)]
            attT_raw = [singles.tile([128, QT], bf16, name=f"attR{i}", tag=f"attR{i}")
                        for i in range(NPAIR)]
            attT = [singles.tile([128, QT], bf16, name=f"attT{i}", tag=f"attT{i}")
                    for i in range(NPAIR)]
            rec_a32 = singles.tile([16, QT], mybir.dt.float32)
            rec_b32 = singles.tile([16, QT], mybir.dt.float32)
            rec_bf_a = singles.tile([16, QT], bf16)
            rec_bf_b = singles.tile([16, QT], bf16)
            xq_sb = [singles.tile([128, QT], bf16, name=f"xq{kk}", tag=f"xq{kk}")
                     for kk in range(16)]

            # =========== Phase 1a: K/V projection (my 256 tokens) + AG ======
            with contextlib.ExitStack() as kv_es:
                kvp = kv_es.enter_context(tc.tile_pool(name="kvp", bufs=1))
                kwork = kv_es.enter_context(tc.tile_pool(name="kwork", bufs=4))
                psA = kv_es.enter_context(tc.tile_pool(name="psA", bufs=2, space="PSUM"))
                psB = kv_es.enter_context(tc.tile_pool(name="psB", bufs=2, space="PSUM"))

                wk_sb = [kvp.tile([128, 512], bf16, name=f"wk{kk}", tag=f"wk{kk}")
                         for kk in range(16)]
                wv_sb = [kvp.tile([128, 512], bf16, name=f"wv{kk}", tag=f"wv{kk}")
                         for kk in range(16)]
                for kk in range(16):
                    r = slice(kk * 128, kk * 128 + 128)
                    nc.sync.dma_start(out=wk_sb[kk], in_=wk[r, :])
                    nc.scalar.dma_start(out=xq_sb[kk], in_=xqT[r, :])
                    nc.scalar.dma_start(out=wv_sb[kk], in_=wv[r, :])

                # K^T = wk^T @ xqT + bk, rope, -> k_in rows m*128..
                for m in range(4):
                    kps = psA.tile([128, QT], mybir.dt.float32, tag="A")
                    for kk in range(16):
                        nc.tensor.matmul(
                            kps, wk_sb[kk][:, m * 128:m * 128 + 128], xq_sb[kk],
                            start=(kk == 0), stop=False)
                    nc.tensor.matmul(
                        kps, bk_sb[:, m * 128:m * 128 + 128], ones_row[:, 0:QT],
                        start=False, stop=True)
                    k_sb = kwork.tile([128, QT], bf16, tag="ksb")
                    nc.vector.tensor_copy(k_sb, kps)
                    swp = psB.tile([128, QT], mybir.dt.float32, tag="B")
                    nc.tensor.matmul(swp, psw_sb, k_sb, start=True, stop=True)
                    t1 = kwork.tile([128, QT], bf16, tag="t1")
                    t2 = kwork.tile([128, QT], bf16, tag="t2")
                    kro = kwork.tile([128, QT], bf16, tag="kro")
                    nc.vector.tensor_mul(t1, k_sb, cq_sb)
                    nc.vector.tensor_mul(t2, swp, dq_sb)
                    nc.vector.tensor_add(kro, t1, t2)
                    nc.gpsimd.dma_start(out=k_in[m * 128:m * 128 + 128, :], in_=kro)
                nc.gpsimd.collective_compute(
                    "AllGather", mybir.AluOpType.bypass, replica_groups=RG,
                    ins=[k_in.opt()], outs=[k_out.opt()])

                # V = xq @ wv + bv (token-major, ones col per kv head) -> v_in
                for blk in range(2):
                    vps = psA.tile([128, 512], mybir.dt.float32, tag="A")
                    for kk in range(16):
                        nc.tensor.matmul(
                            vps, xq_sb[kk][:, blk * 128:blk * 128 + 128], wv_sb[kk],
                            start=(kk == 0), stop=False)
                    nc.tensor.matmul(
                        vps, ones_row[:, 0:128], bv_sb, start=False, stop=True)
                    vloc = kwork.tile([128, N_KV * 65], bf16, tag="vloc")
                    nc.vector.memset(vloc, 1.0)
                    for h in range(N_KV):
                        nc.vector.tensor_copy(
                            vloc[:, h * 65:h * 65 + 64], vps[:, h * 64:h * 64 + 64])
                    nc.gpsimd.dma_start(out=v_in[blk * 128:blk * 128 + 128, :], in_=vloc)
                nc.gpsimd.collective_compute(
                    "AllGather", mybir.AluOpType.bypass, replica_groups=RG,
                    ins=[v_in.opt()], outs=[v_out.opt()])

            # ---- load gathered K/V into SBUF (rank-slot kv tile order) ----
            kog = k_out[:].rearrange("(r kh p) c -> kh p r c", r=4, kh=N_KV, p=64)
            for kvh in range(N_KV):
                nc.sync.dma_start(out=ropek2[kvh], in_=kog[kvh])
            vog = v_out[:].rearrange("(r s p) c -> (r s) p c", r=4, s=2, p=128)
            for t in range(NT):
                nc.sync.dma_start(out=vall[t], in_=vog[t])

            # =========== Phase 1b: Q projection + rope ===========
            with contextlib.ExitStack() as q_es:
                qwork = q_es.enter_context(tc.tile_pool(name="qwork", bufs=4))
                wqpool = q_es.enter_context(tc.tile_pool(name="wqp", bufs=3))
                psQ = q_es.enter_context(tc.tile_pool(name="psQ", bufs=2, space="PSUM"))
                psR = q_es.enter_context(tc.tile_pool(name="psR", bufs=2, space="PSUM"))

                for m in range(16):
                    wqm = wqpool.tile([128, 16, 128], bf16, tag="wqm")
                    nc.scalar.dma_start(
                        out=wqm,
                        in_=wq[:, m * 128:m * 128 + 128].rearrange(
                            "(kk p) c -> p kk c", p=128))
                    qps = psQ.tile([128, QT], mybir.dt.float32, tag="Q")
                    for kk in range(16):
                        nc.tensor.matmul(
                            qps, wqm[:, kk, :], xq_sb[kk],
                            start=(kk == 0), stop=False)
                    nc.tensor.matmul(
                        qps, bq_sb[:, m * 128:m * 128 + 128], ones_row[:, 0:QT],
                        start=False, stop=True)
                    q_sb = qwork.tile([128, QT], bf16, tag="qsb")
                    nc.vector.tensor_copy(q_sb, qps)
                    swq = psR.tile([128, QT], mybir.dt.float32, tag="R")
                    nc.tensor.matmul(swq, psw_sb, q_sb, start=True, stop=True)
                    t1 = qwork.tile([128, QT], bf16, tag="qt1")
                    t2 = qwork.tile([128, QT], bf16, tag="qt2")
                    nc.vector.tensor_mul(t1, q_sb, cq_sb)
                    nc.vector.tensor_mul(t2, swq, dq_sb)
                    nc.vector.tensor_add(ropeq[2 * m], t1[0:64, :], t2[0:64, :])
                    nc.vector.tensor_add(ropeq[2 * m + 1], t1[64:128, :], t2[64:128, :])

            # =========== Phase 2: attention + Phase 3: out projection ======
            with contextlib.ExitStack() as att_es:
                probsp = att_es.enter_context(tc.tile_pool(name="probsp", bufs=2))
                unp = att_es.enter_context(tc.tile_pool(name="unp", bufs=2))
                psS = att_es.enter_context(tc.tile_pool(name="psS", bufs=2, space="PSUM"))
                psM = att_es.enter_context(tc.tile_pool(name="psM", bufs=2, space="PSUM"))
                psD = att_es.enter_context(tc.tile_pool(name="psD", bufs=1, space="PSUM"))
                wopool = att_es.enter_context(tc.tile_pool(name="wop", bufs=4))
                opool = att_es.enter_context(tc.tile_pool(name="osb", bufs=3))

                psd_a = psD.tile([16, QT], mybir.dt.float32, tag="Da")
                psd_b = psD.tile([16, QT], mybir.dt.float32, tag="Db")

                # prefetch all of wo early on the (now idle) gpsimd queue
                wons = []
                for n in range(4):
                    won = wopool.tile([128, 16, 512], bf16, tag="won")
                    nc.gpsimd.dma_start(
                        out=won,
                        in_=wo[:, n * 512:n * 512 + 512].rearrange(
                            "(p q) c -> q p c", q=128))
                    wons.append(won)

                def normalize(pr):
                    rec = rec_bf_a if pr < NPAIR // 2 else rec_bf_b
                    bcp = psM.tile([128, 512], mybir.dt.float32, tag="M")
                    nc.tensor.matmul(
                        bcp[:, 0:QT], selp_sb[:, pr * 128:pr * 128 + 128], rec,
                        start=True, stop=True)
                    nc.vector.tensor_mul(attT[pr], attT_raw[pr], bcp[:, 0:QT])

                probs_q = [None] * NPAIR

                def emit_scores(pr):
                    kvh = pr // 2
                    probs = probsp.tile([128, NT, 2, QT], bf16, tag="P")
                    probs_q[pr] = probs
                    for sub in range(2):
                        h = 2 * pr + sub
                        for half in range(2):
                            pss = psS.tile([128, 4, QT], mybir.dt.float32, tag="S")
                            for tt in range(4):
                                t = half * 4 + tt
                                nc.tensor.matmul(
                                    pss[:, tt, :],
                                    ropek2[kvh][:, t // 2, (t % 2) * 128:(t % 2) * 128 + 128],
                                    ropeq[h],
                                    start=True, stop=True)
                            pv = probs[:, half * 4:half * 4 + 4, sub, :]
                            nc.scalar.activation(
                                pv, pss,
                                mybir.ActivationFunctionType.Exp,
                                bias=0.0, scale=0.125)
                            nc.vector.tensor_mul(
                                pv, pv, mask_sb[:, half * 4:half * 4 + 4, :])

                def emit_av(pr):
                    kvh = pr // 2
                    probs = probs_q[pr]
                    half2 = pr >= NPAIR // 2
                    psd = psd_b if half2 else psd_a
                    # AV with pair-shared stationary V (N=512: [A|B])
                    psv = psM.tile([128, 512], mybir.dt.float32, tag="M")
                    for t in range(NT):
                        nc.tensor.matmul(
                            psv[0:65, :], vall[t][:, kvh * 65:kvh * 65 + 65],
                            probs[:, t, :, :],
                            start=(t == 0), stop=(t == NT - 1))
                    unnorm = unp.tile([65, 512], bf16, tag="U")
                    nc.vector.tensor_copy(unnorm, psv[0:65, :])
                    # denominators (row 64) -> psd rows {2pr, 2pr+1} (mod 16)
                    base = NPAIR // 2 * 2 if half2 else 0
                    for sub in range(2):
                        h = 2 * pr + sub
                        nc.tensor.matmul(
                            psd, dsel_sb[64:65, h * 16:h * 16 + 16],
                            unnorm[64:65, sub * QT:sub * QT + QT],
                            start=(h % 16 == 0), stop=(h % 16 == 15))
                    # pack the two heads' [64, 256] into [128, 256]
                    pk = psM.tile([128, 512], mybir.dt.float32, tag="M")
                    nc.tensor.matmul(pk[:, 0:QT], pka_sb, unnorm[0:64, 0:QT],
                                     start=True, stop=False)
                    nc.tensor.matmul(pk[:, 0:QT], pkb_sb, unnorm[0:64, QT:512],
                                     start=False, stop=True)
                    nc.vector.tensor_copy(attT_raw[pr], pk[:, 0:QT])

                # software pipeline: scores/exp of pair p+1 overlap AV of pair p
                for pr in range(NPAIR + 1):
                    if pr < NPAIR:
                        emit_scores(pr)
                    if pr >= 1:
                        p = pr - 1
                        emit_av(p)
                        if p == NPAIR // 2 - 1:
                            nc.vector.reciprocal_approx_fast(rec_a32, psd_a)
                            nc.vector.tensor_copy(rec_bf_a, rec_a32)
                            for p2 in range(NPAIR // 2):
                                normalize(p2)
                        elif p == NPAIR - 1:
                            nc.vector.reciprocal_approx_fast(rec_b32, psd_b)
                            nc.vector.tensor_copy(rec_bf_b, rec_b32)
                            for p2 in range(NPAIR // 2, NPAIR):
                                normalize(p2)

                # =========== Phase 3: output projection ===========
                for n in range(4):
                    for blk in range(2):
                        ops = psM.tile([128, 512], mybir.dt.float32, tag="M")
                        for p in range(16):
                            nc.tensor.matmul(
                                ops, attT[p][:, blk * 128:blk * 128 + 128],
                                wons[n][:, p, :],
                                start=(p == 0), stop=False)
                        nc.tensor.matmul(
                            ops, ones_row[:, 0:128], bo_sb[:, n * 512:n * 512 + 512],
                            start=False, stop=True)
                        osb = opool.tile([128, 512], mybir.dt.float32, tag="osb")
                        nc.vector.tensor_copy(osb, ops)
                        nc.sync.dma_start(
                            out=out[blk * 128:blk * 128 + 128, n * 512:n * 512 + 512],
                            in_=osb)
    return nc


def _host_prep(x, Wq, bq, Wk, bk, Wv, bv, Wo, bo):
    """Build per-core input maps."""
    # per-head even/odd deinterleave permutation of output columns
    def colperm(nheads):
        p = []
        for h in range(nheads):
            base = h * DK
            p.extend([base + 2 * j for j in range(HALF)])
            p.extend([base + 2 * j + 1 for j in range(HALF)])
        return np.array(p)

    qperm = colperm(N_HEAD)
    kperm = colperm(N_KV)
    wq_p = np.ascontiguousarray(Wq[:, qperm]).astype(BF16)
    wk_p = np.ascontiguousarray(Wk[:, kperm]).astype(BF16)
    bq_p = np.ascontiguousarray(bq[qperm]).astype(BF16).reshape(1, D_MODEL)
    bk_p = np.ascontiguousarray(bk[kperm]).astype(BF16).reshape(1, 512)
    wv_c = Wv.astype(BF16)
    wo_c = Wo.astype(BF16)
    bv_r = bv.astype(BF16).reshape(1, 512)
    bo_r = bo.astype(BF16).reshape(1, D_MODEL)

    invf = THETA ** (-(np.arange(HALF, dtype=np.float64) * 2.0 / DK))
    posf = np.arange(S, dtype=np.float64)
    ang = posf[:, None] * invf[None, :]  # [S, 32]
    cos_t, sin_t = np.cos(ang), np.sin(ang)

    def rope_tables(pos_idx):
        # [128, len(pos_idx)] tables in deinterleaved space (2 heads / 128 rows)
        n = len(pos_idx)
        C = np.zeros((128, n), np.float32)
        D = np.zeros((128, n), np.float32)
        for p in range(128):
            r = p % DK
            i = r if r < HALF else r - HALF
            C[p] = cos_t[pos_idx, i]
            D[p] = (-sin_t[pos_idx, i]) if r < HALF else sin_t[pos_idx, i]
        return C.astype(BF16), D.astype(BF16)

    psw = np.zeros((128, 128), np.float32)
    for m in range(128):
        k = m + HALF if (m % DK) < HALF else m - HALF
        psw[k, m] = 1.0
    psw = psw.astype(BF16)
    pka = np.zeros((64, 128), np.float32)
    pkb = np.zeros((64, 128), np.float32)
    for k in range(64):
        pka[k, k] = 1.0
        pkb[k, k + 64] = 1.0
    pka, pkb = pka.astype(BF16), pkb.astype(BF16)

    dsel = np.zeros((65, N_HEAD * 16), np.float32)
    for h in range(N_HEAD):
        dsel[64, h * 16 + (h % 16)] = 1.0
    dsel = dsel.astype(BF16)
    selp = np.zeros((16, NPAIR * 128), np.float32)
    for pr in range(NPAIR):
        selp[(2 * pr) % 16, pr * 128:pr * 128 + 64] = 1.0
        selp[(2 * pr + 1) % 16, pr * 128 + 64:pr * 128 + 128] = 1.0
    selp = selp.astype(BF16)

    # kv tile t (rank-slot order) covers q block: r if slot 0 else 7-r
    kvblock = [(t // 2) if (t % 2 == 0) else 7 - (t // 2) for t in range(NT)]

    in_maps = []
    meta = []
    for c in range(8):
        b, j = c // 4, c % 4
        blocks = [j, 7 - j]
        qrows = np.concatenate([np.arange(bb * 128, bb * 128 + 128) for bb in blocks])
        xb = np.asarray(x[b], dtype=np.float32)
        xqT = np.ascontiguousarray(xb[qrows].T).astype(BF16)
        cqt, dqt = rope_tables(qrows)
        # mask[kt, t*QT + blk*128 + ql] = 1 if kv_pos <= q_pos else 0
        mask = np.zeros((128, NT * QT), np.float32)
        kt_local = np.arange(128)
        for t in range(NT):
            ktg = kvblock[t] * 128 + kt_local
            for blki, bb in enumerate(blocks):
                qpos = bb * 128 + np.arange(128)
                mask[:, t * QT + blki * 128:t * QT + blki * 128 + 128] = (
                    ktg[:, None] <= qpos[None, :]
                )
        in_maps.append({
            "xqT": xqT, "wq": wq_p, "wk": wk_p, "wv": wv_c, "wo": wo_c,
            "bqr": bq_p, "bkr": bk_p, "bvr": bv_r, "bor": bo_r,
            "cqt": cqt, "dqt": dqt,
            "pswap": psw, "packa": pka, "packb": pkb,
            "dsel": dsel, "selp": selp,
            "maskT": mask.astype(BF16),
        })
        meta.append((b, blocks))
    return in_maps, meta


def kernel(x, Wq, bq, Wk, bk, Wv, bv, Wo, bo):
    if "nc" not in _cache:
        nc0 = _build_nc()
        nc0.finalize()
        _cache["nc"] = nc0
    nc = _cache["nc"]
    in_maps, meta = _host_prep(x, Wq, bq, Wk, bk, Wv, bv, Wo, bo)
    res = run_bass_kernel_spmd(nc, in_maps, list(range(8)))
    full = np.zeros((B, S, D_MODEL), np.float32)
    for c in range(8):
        b, blocks = meta[c]
        o = res.results[c]["out"]
        for blki, bb in enumerate(blocks):
            full[b, bb * 128:bb * 128 + 128] = o[blki * 128:(blki + 1) * 128]
    return full
